# revision 47
# baseline (speedup 1.0000x reference)
"""AdaptiveTripletLoss on 8 TRN2 NeuronCores.

Device: the compute-dominant Gram matrix G = E @ E^T (4096x4096x2048)
in fp8 DoubleRow on the PE, f32 PSUM. Exact-cover symmetric
assignment: each core holds 4 blocks [A, C, B, D] plus a 256-col
weight sliver (HW) and computes 4 full block-pairs
(A,A),(A,C),(A,D),(B,C) and one half pair (v,B) — the 4 antipodal
pairs are split by output-row halves between two cores, with the
m-range baked into the host-packed HW region so the program is SPMD.
144 matmuls/core (vs 160 with padded assignments), 4.5 MB streamed
input. The 18 (slot, m) accumulation units run in three PSUM
cohorts (8/8/2); cohort 1 consumes the four streamed chunks as they
arrive. Dummy matmuls pre-warm the PE clock (HAM) during the first
chunk's DMA; redundant Ldweights are deduped post-trace. Outputs
drain per cohort as batched DMAs with 4 KB contiguous dram runs.
Host mirrors the blocks, then does masks/counts, order-statistic
selection, exact d_ap/d_an norms and the masked mean.
"""

import os

import numpy as np
import ml_dtypes

N, D = 4096, 2048
NUM_IDS = 512
N_CORES = 8
MARGIN = 0.3
RATIOS = (0.3, 0.4, 0.3)
EPS = 1e-6

B = 512           # block edge
NB = N // B       # 8x8 block grid
KT = D // 128     # 16 k-tiles
TT = KT // 2      # 8 DoubleRow steps, each contracting 256 k-rows
KC = 4            # input DMA chunks (each = 2 DoubleRow steps)
N_DUMMY = 5       # PE pre-warm matmuls during the first chunk's DMA
NU = 18           # output units: 16 full (slot,m) + 2 half-slot steps

# Exact-cover assignment: per core, blocks [A, C, B, D], half pair
# (v, B) with output rows m in {mbase, mbase+1}. Together the 8 cores
# cover all 36 unordered block pairs exactly once (antipodal pairs
# {0,4},{1,5},{2,6},{3,7} split between two cores by m-half).
ASSIGN = [
    {'A': 4, 'C': 1, 'B': 7, 'D': 3, 'v': 3, 'mbase': 0},
    {'A': 0, 'C': 5, 'B': 4, 'D': 2, 'v': 0, 'mbase': 0},
    {'A': 5, 'C': 2, 'B': 4, 'D': 3, 'v': 0, 'mbase': 2},
    {'A': 2, 'C': 3, 'B': 6, 'D': 1, 'v': 2, 'mbase': 0},
    {'A': 7, 'C': 4, 'B': 6, 'D': 2, 'v': 2, 'mbase': 2},
    {'A': 3, 'C': 0, 'B': 7, 'D': 1, 'v': 3, 'mbase': 2},
    {'A': 6, 'C': 7, 'B': 5, 'D': 0, 'v': 1, 'mbase': 0},
    {'A': 1, 'C': 6, 'B': 5, 'D': 0, 'v': 1, 'mbase': 2},
]

LAST_EXEC_NS = None


def _dedupe_ldweights(nc):
    """Remove Ldweights instructions identical to the immediately
    preceding one (same weights AP + mode): the PE array keeps the
    stationary operand loaded across matmuls, so consecutive matmuls
    sharing lhsT only need the first load."""
    removed = 0
    for fn in nc.m.functions:
        for blk in fn.blocks:
            il = blk.instructions
            prev_key = None
            prev_sync = None
            dels = []
            for idx in range(len(il)):
                ins = il[idx]
                if ins.opcode == "Ldweights":
                    ap = ins.ins[0]
                    key = (
                        getattr(ap, "memref", None),
                        ap.offset,
                        str(ap.ap),
                        str(ins.perf_mode),
                        str(getattr(ins, "is_transpose", None)),
                        str(getattr(ins, "tile_position", None)),
                    )
                    sync = tuple(sorted(ins.sync_dependency_names()))
                    nosync = tuple(sorted(ins.nosync_dependency_names()))
                    if key == prev_key and sync == prev_sync and not nosync:
                        dels.append(idx)
                    else:
                        prev_key = key
                        prev_sync = sync
            for idx in reversed(dels):
                del il[idx]
            removed += len(dels)
    return removed


def _hoist_head_dmas(nc, n=5):
    """Move the first n input DMACopy issues (sync ring) into the
    preamble block: they have no dependencies, so the first chunks
    stream in during the ~7us engine-init barrier instead of after it."""
    fn = nc.m.functions[0]
    blk0, blk1 = fn.blocks[0], fn.blocks[1]
    il1 = blk1.instructions
    picked = []
    for idx in range(len(il1)):
        ins = il1[idx]
        if ins.opcode == "DMACopy" and "SP" in str(ins.engine):
            if ins.sync_dependency_names() or ins.nosync_dependency_names():
                break
            picked.append(idx)
            if len(picked) == n:
                break
    moved = []
    for idx in reversed(picked):
        moved.append(il1[idx])
        del il1[idx]
    moved.reverse()
    il0 = blk0.instructions
    pos = 1
    for ins in moved:
        il0.insert(pos, ins)
        pos += 1
    return len(moved)


def _build_gram_kernel():
    import concourse.bacc as bacc
    import concourse.tile as tile
    from concourse import mybir

    nc = bacc.Bacc(None, target_bir_lowering=False)

    f32 = mybir.dt.float32
    bf16 = mybir.dt.bfloat16
    fp8 = mybir.dt.float8e4
    DR = mybir.MatmulPerfMode.DoubleRow

    # Full blocks split by cohort need: cohort 1 reads only A, C; the
    # B, D halves stream after all A/C chunks so cohort 1 is PE-bound
    # from the first chunk on.
    blksAC = nc.declare_dram_parameter("blksAC", [KC * 128, 2 * 2048], fp8,
                                       isOutput=False)
    blksBD = nc.declare_dram_parameter("blksBD", [KC * 128, 2 * 2048], fp8,
                                       isOutput=False)
    # HW weight sliver: [KC, 2(u), 2(i), 256] per partition = 4096 B.
    hwP = nc.declare_dram_parameter("hwP", [128, KC * 1024], fp8,
                                    isOutput=False)
    # Output: row p holds all 18 units' 512 cols (unit-major): a 4-unit
    # drain DMA writes 4 KB contiguous runs per dram row.
    out = nc.declare_dram_parameter("out", [128, NU * B], bf16,
                                    isOutput=True)

    with tile.TileContext(nc) as tc:
        with (
            tc.tile_pool(name="data", bufs=1) as dp,
            tc.tile_pool(name="warm", bufs=1) as wp,
            tc.tile_pool(name="psum", bufs=8, space="PSUM") as pp,
            tc.tile_pool(name="outp", bufs=3) as op,
        ):
            data = dp.tile([128, KC, 4, 2, 2, 512], fp8, name="data")
            hw = dp.tile([128, KC, 2, 2, 256], fp8, name="hw")
            # A/C stream first (cohort 1), chunk 0 split A then C for the
            # earliest possible start; then B/D (cohort 2), then HW.
            # A tiny leading DMA absorbs the ring's ~1.4us cold-start lag
            # so the A sub-chunk lands earlier.
            nc.sync.dma_start(hw[:, 0, 0, 0, 0:64], hwP[0:128, 0:64])
            nc.sync.dma_start(data[:, 0, 0, 0, :, :], blksAC[0:128, 0:1024])
            nc.sync.dma_start(data[:, 0, 0, 1, :, :],
                              blksAC[0:128, 1024:2048])
            nc.sync.dma_start(data[:, 0, 1, :, :, :],
                              blksAC[0:128, 2048:4096])
            for k in range(1, KC):
                nc.sync.dma_start(data[:, k, 0:2, :, :, :],
                                  blksAC[k * 128:(k + 1) * 128, :])
            for k in range(KC):
                nc.sync.dma_start(data[:, k, 2:4, :, :, :],
                                  blksBD[k * 128:(k + 1) * 128, :])
            nc.sync.dma_start(hw[:, :, :, :, :], hwP[:, :])

            # PE pre-warm: dummy matmuls on a zeroed tile while chunk 0
            # streams in; keeps HAM from throttling the first real mms.
            wl = wp.tile([128, 2, 128], fp8, name="wl")
            wr = wp.tile([128, 2, 512], fp8, name="wr")
            nc.vector.memset(wl[:], 0.0)
            nc.vector.memset(wr[:], 0.0)
            wps = pp.tile([128, B], f32, name="ps")
            for _ in range(N_DUMMY):
                nc.tensor.matmul(wps[:], wl[:], wr[:], start=True, stop=True,
                                 perf_mode=DR)

            # units: 0-3 f0=(A,A) m0-3; 4-7 f1=(A,C); 8-11 f2=(A,D);
            # 12-15 f3=(B,C); 16-17 h_s=(v,B) packed m-halves.
            ps = {}

            def mm(unit, wq, rq, m, t, start, stop, hws=None, c0=0):
                # c0: first rhs column (diagonal blocks are symmetric, so
                # f0's matmul m only needs cols >= m*128; host mirrors).
                k, u = t // 2, t % 2
                if hws is None:
                    w_ap = data[:, k, wq, u, :, m * 128:(m + 1) * 128]
                else:
                    w_ap = hw[:, k, u, :, hws * 128:(hws + 1) * 128]
                nc.tensor.matmul(
                    ps[unit][:, c0:], w_ap, data[:, k, rq, u, :, c0:],
                    start=start, stop=stop, perf_mode=DR,
                )

            cast_i = 0

            def drain(units, ring=None, split=False):
                """Cast psums to bf16 and DMA out in one batched
                transfer (4 KB contiguous per dram row). split=True
                halves a cast across vector+scalar so the PSUM bank
                frees sooner (used where the next cohort waits)."""
                nonlocal cast_i
                g = op.tile([128, len(units), B], bf16, name="ot")
                for i, u in enumerate(units):
                    if split:
                        nc.vector.tensor_copy(g[:, i, 0:256],
                                              ps[u][:, 0:256])
                        nc.scalar.copy(g[:, i, 256:512], ps[u][:, 256:512])
                        continue
                    eng = (nc.vector, nc.scalar)[cast_i % 2]
                    if eng is nc.scalar:
                        eng.copy(g[:, i, :], ps[u][:])
                    else:
                        eng.tensor_copy(g[:, i, :], ps[u][:])
                    cast_i += 1
                u0 = units[0]
                (ring or nc.scalar).dma_start(
                    out[:, u0 * B:(u0 + len(units)) * B], g[:])

            # Cohorts of 7/7/4 units: one PSUM bank is always spare at a
            # cohort boundary, so the next cohort's first unit starts
            # without waiting for a drain cast.
            # Cohort 1: f0 (A,A) units 0-3, f1 (A,C) m0-2 units 4-6;
            # consume chunks in arrival order. Phase 0 runs all f0
            # matmuls before f1 so the A-only prefix starts as soon as
            # the A sub-chunk lands (C arrives ~0.7us later).
            for u in range(7):
                ps[u] = pp.tile([128, B], f32, name="ps")
            for c in range(KC):
                for tt in (2 * c, 2 * c + 1):
                    for m in range(4):
                        st, sp = (tt == 0), (tt == TT - 1)
                        mm(0 + m, 0, 0, m, tt, st, sp, c0=m * 128)
                        if c > 0 and m < 3:
                            mm(4 + m, 0, 1, m, tt, st, sp)
                if c == 0:
                    for tt in (0, 1):
                        for m in range(3):
                            mm(4 + m, 0, 1, m, tt, (tt == 0), False)
            drain([0, 1, 2, 3], split=True)
            drain([4, 5, 6], split=True)

            # Cohort 2: f1 m3 (unit 7), f2 (A,D) units 8-11, f3 (B,C)
            # m0,m1 units 12,13. First-use order matches the bank
            # rotation so each unit's bank cast is already done.
            for u in range(7, 14):
                ps[u] = pp.tile([128, B], f32, name="ps")
            for t in range(TT):
                st, sp = (t == 0), (t == TT - 1)
                mm(7, 0, 1, 3, t, st, sp)
                mm(8, 0, 3, 0, t, st, sp)
                mm(9, 0, 3, 1, t, st, sp)
                mm(10, 0, 3, 2, t, st, sp)
                mm(12, 2, 1, 0, t, st, sp)
                mm(13, 2, 1, 1, t, st, sp)
                mm(11, 0, 3, 3, t, st, sp)
            drain([7, 8, 9, 10], ring=nc.sync, split=True)
            drain([11, 12, 13])

            # Cohort 3: f3 m2,m3 units 14,15 and half-slot steps h0, h1
            # (v,B) units 16,17; drains split across both DMA rings to
            # shorten the tail.
            for u in range(14, 18):
                ps[u] = pp.tile([128, B], f32, name="ps")
            for t in range(TT):
                st, sp = (t == 0), (t == TT - 1)
                mm(14, 2, 1, 2, t, st, sp)
                mm(15, 2, 1, 3, t, st, sp)
                mm(16, None, 2, None, t, st, sp, hws=0)
                mm(17, None, 2, None, t, st, sp, hws=1)
            drain([14, 15])
            drain([16], ring=nc.sync)
            # last unit: cast split across both engines; scalar issues
            # its half's DMA right after its own cast (engine-local
            # ordering), sync picks up vector's half.
            g17 = op.tile([128, B], bf16, name="ot")
            nc.scalar.copy(g17[:, 0:256], ps[17][:, 0:256])
            nc.scalar.dma_start(out[:, 17 * B:17 * B + 256], g17[:, 0:256])
            nc.vector.tensor_copy(g17[:, 256:512], ps[17][:, 256:512])
            nc.sync.dma_start(out[:, 17 * B + 256:18 * B], g17[:, 256:512])

    _dedupe_ldweights(nc)
    nc.compile()
    return nc


_NC_CACHE = None


def _pack_core(eT8: np.ndarray, w: int):
    """Pack core w's blocks [A, C, B, D] and HW sliver into the device
    layouts. Row k*128+p of blks holds [q, u, i, c] with k-row index
    (4k + 2u + i)*128 + p of eT."""
    g = ASSIGN[w]
    R = eT8.reshape(KC, 2, 2, 128, NB, B)  # [k, u, i, p, block, c]

    def pack2(q0, q1):
        A = R[:, :, :, :, [q0, q1], :]      # [KC, 2, 2, 128, 2, B]
        A = A.transpose(0, 3, 4, 1, 2, 5)   # [KC, 128, 2, 2, 2, B]
        return np.ascontiguousarray(A).reshape(KC * 128, 2 * 2048)

    blks_ac = pack2(g['A'], g['C'])
    blks_bd = pack2(g['B'], g['D'])
    # HW: block v columns mbase*128 .. (mbase+2)*128 -> [p, KC, u, i, 256]
    H = R[:, :, :, :, g['v'], g['mbase'] * 128:(g['mbase'] + 2) * 128]
    H = H.transpose(3, 0, 1, 2, 4)      # [128, KC, 2, 2, 256]
    hwp = np.ascontiguousarray(H).reshape(128, KC * 1024)
    return blks_ac, blks_bd, hwp


def _run_gram(emb: np.ndarray) -> np.ndarray:
    """Run the 8-core symmetric Gram kernel; returns G = emb @ emb.T f32."""
    global _NC_CACHE, LAST_EXEC_NS
    from concourse.bass_utils import run_bass_kernel_spmd

    if _NC_CACHE is None:
        _NC_CACHE = _build_gram_kernel()
    nc = _NC_CACHE

    eT8 = np.ascontiguousarray(emb.T).astype(ml_dtypes.float8_e4m3)
    in_maps = []
    for w in range(N_CORES):
        blks_ac, blks_bd, hwp = _pack_core(eT8, w)
        in_maps.append({"blksAC": blks_ac, "blksBD": blks_bd, "hwP": hwp})

    trace = bool(int(os.environ.get("KERNEL_TRACE", "0")))
    res = run_bass_kernel_spmd(
        nc, in_maps, core_ids=list(range(N_CORES)), trace=trace
    )
    if res.exec_time_ns is not None:
        LAST_EXEC_NS = res.exec_time_ns

    G = np.empty((N, N), dtype=np.float32)
    for w in range(N_CORES):
        g = ASSIGN[w]
        o = np.asarray(res.results[w]["out"], dtype=np.float32)
        o = o.reshape(128, NU, B)
        slot_pairs = [(g['A'], g['A']), (g['A'], g['C']),
                      (g['A'], g['D']), (g['B'], g['C'])]
        for ui in range(NU):
            if ui < 16:
                wr, rc = slot_pairs[ui // 4]
                m = ui % 4
            else:
                wr, rc = g['v'], g['B']
                m = g['mbase'] + (ui - 16)
            rows = slice(wr * B + m * 128, wr * B + (m + 1) * 128)
            blk = o[:, ui, :]
            if ui < 4:
                # diagonal block: device computed only cols >= m*128;
                # mirror the rectangle into both triangles.
                sub = blk[:, m * 128:]
                G[rows, rc * B + m * 128:(rc + 1) * B] = sub
                G[rc * B + m * 128:(rc + 1) * B, rows] = sub.T
            else:
                G[rows, rc * B:(rc + 1) * B] = blk
                if rc != wr:
                    G[rc * B:(rc + 1) * B, rows] = blk.T
    return G


def _sample_js(counts: np.ndarray, us: list) -> np.ndarray:
    """Replicate the reference's f32 sampling math. counts [N] int, us 3x[N]
    f32 uniforms. Returns j ranks [N, 3] int64 (rank into the masked sort)."""
    out = []
    for t, r in enumerate(RATIOS):
        cnt = np.maximum(
            np.int32(1),
            np.floor(counts.astype(np.float32) * np.float32(r)).astype(np.int32),
        )
        j = np.minimum((us[t] * cnt.astype(np.float32)).astype(np.int32), cnt - 1)
        out.append(j.astype(np.int64))
    return np.stack(out, axis=1)


def kernel(embeddings: np.ndarray, labels: np.ndarray) -> np.ndarray:
    emb = np.ascontiguousarray(np.asarray(embeddings, dtype=np.float32))
    lab = np.asarray(labels).astype(np.int64)

    G = _run_gram(emb)

    # Selection keys: within row i, ordering by (sq_j - 2 G[i,j]) equals
    # ordering by distance.
    sq = np.einsum("ij,ij->i", emb, emb).astype(np.float32)

    # Uniforms must match jax.random with key 42 bit-exactly.
    import jax

    with jax.default_device(jax.devices("cpu")[0]):
        skey = jax.random.key(42)
        keys = jax.random.split(skey, 6)
        us = [np.asarray(jax.random.uniform(k, (N,))) for k in keys]

    class_size = np.bincount(lab, minlength=NUM_IDS)
    pos_count = class_size[lab] - 1
    neg_count = N - class_size[lab]
    valid = (pos_count > 0) & (neg_count > 0)

    pos_js = _sample_js(pos_count, us[0:3])  # [N, 3]
    neg_js = _sample_js(neg_count, us[3:6])  # [N, 3]

    # Per-class member lists
    order = np.argsort(lab, kind="stable")
    sorted_lab = lab[order]
    starts = np.searchsorted(sorted_lab, np.arange(NUM_IDS), side="left")
    ends = np.searchsorted(sorted_lab, np.arange(NUM_IDS), side="right")

    pos_idx = np.zeros((N, 3), dtype=np.int64)
    neg_idx = np.zeros((N, 3), dtype=np.int64)
    INF = np.float32(np.inf)

    for i in range(N):
        li = lab[i]
        members = order[starts[li]:ends[li]]
        key_row = sq - 2.0 * G[i]  # f32 [N]
        if valid[i]:
            pos_members = members[members != i]
            pk = key_row[pos_members]
            po = np.argsort(pk, kind="stable")
            pos_idx[i] = pos_members[po[pos_js[i]]]
        # negatives: mask out own class and self
        nk = key_row.copy()
        nk[members] = INF
        nk[i] = INF
        kth = np.unique(neg_js[i])
        part = np.argpartition(nk, kth)
        neg_idx[i] = part[neg_js[i]]

    a = emb[:, None, :]
    p = emb[pos_idx]
    ng = emb[neg_idx]
    d_ap = np.sqrt(np.sum((a - p + np.float32(EPS)) ** 2, axis=-1))
    d_an = np.sqrt(np.sum((a - ng + np.float32(EPS)) ** 2, axis=-1))
    tri = np.maximum(d_ap - d_an + np.float32(MARGIN), np.float32(0.0))
    w = valid[:, None].astype(np.float32)
    denom = max(3.0 * float(valid.sum()), 1.0)
    loss = np.float32(np.sum(tri * w) / denom)
    return np.array(loss, dtype=np.float32)
